# revision 12
# baseline (speedup 1.0000x reference)
"""GMM E-step (log responsibilities) on 8 Trainium2 NeuronCores.

Math per sample x_n and component k (D=256, K=64, N=131072):
    y      = x @ L_k - mu_k @ L_k                (L_k = precision cholesky)
    sq     = ||y||^2
           = ||x L_k||^2 - 2 x . (L_k @ t_k) + ||t_k||^2,   t_k = mu_k @ L_k
    wlp    = -0.5*(D log 2pi + sq) + log|L_k| + log w_k
    lpn    = logsumexp_k wlp
    out    = (mean(lpn), wlp - lpn[:, None])

Data parallel over samples: each core gets 16384 rows. Small GMM params are
replicated. Device computes, per 128-row tile:
  - PSUM pair tiles  [128, 512] = X_tile @ [L_k | L_{k+1}]  (fp32r matmuls,
    full PE rate since the moving free dim is 512 >= 256)
  - psum_t [128, 64] = X_tile @ Cmat (+0 col trick), Cmat[:, k] = L_k @ t_k
  - halfsq[:, k]     = 0.5 * sum_j Y_kj^2  (ScalarE Square+accum for even k,
    VectorE tensor_tensor_reduce for odd k - keeps both engines under TensorE)
  - wlp = psum_t - halfsq + const ; then logsumexp + output tile.
"""

import os
import sys

import numpy as np

for _p in ("/opt/trn_rl_repo",):
    if _p not in sys.path and os.path.isdir(_p):
        sys.path.insert(0, _p)

N, K, D = 131072, 64, 256
NCORES = 8
NS = N // NCORES          # 16384 rows per core
P = 128                   # partition dim / row tile
NT = NS // P              # 128 row tiles per core
DC = D // P               # 2 contraction chunks
NPAIR = K // 2            # 32 component pairs

_CACHE = {}


def _build_program():
    import concourse.bass as bass
    import concourse.tile as tile
    from concourse import bacc, mybir

    f32 = mybir.dt.float32
    f32r = mybir.dt.float32r
    AF = mybir.ActivationFunctionType
    ALU = mybir.AluOpType

    nc = bacc.Bacc("TRN2", target_bir_lowering=False, debug=False,
                   num_devices=NCORES)

    XT = nc.declare_dram_parameter("XT", (D, NS), f32r, isOutput=False)
    B = nc.declare_dram_parameter("B", (DC, P, K * D), f32r, isOutput=False)
    C = nc.declare_dram_parameter("C", (DC, P, K), f32r, isOutput=False)
    CONST = nc.declare_dram_parameter("CONST", (1, K), f32, isOutput=False)
    RESP = nc.declare_dram_parameter("RESP", (NS, K), f32, isOutput=True)
    LPN = nc.declare_dram_parameter("LPN", (P, 1), f32, isOutput=True)

    sqrt_half = float(np.sqrt(0.5))

    with tile.TileContext(nc) as tc:
        with (
            tc.tile_pool(name="singles", bufs=1) as singles,
            tc.tile_pool(name="xt", bufs=3) as xt_pool,
            tc.tile_pool(name="scr", bufs=4) as scr_pool,
            tc.tile_pool(name="small", bufs=3) as small_pool,
            tc.tile_pool(name="resp", bufs=3) as resp_pool,
            tc.tile_pool(name="ppair", bufs=6, space="PSUM") as ppair_pool,
            tc.tile_pool(name="pt", bufs=2, space="PSUM") as pt_pool,
        ):
            # ---- resident tensors ----
            b_sb = singles.tile([P, DC, K * D], f32r)
            for c in range(DC):
                nc.sync.dma_start(out=b_sb[:, c, :], in_=B[c])
            c_sb = singles.tile([P, DC, K], f32r)
            for c in range(DC):
                nc.sync.dma_start(out=c_sb[:, c, :], in_=C[c])
            const_sb = singles.tile([P, K], f32)
            const_bcast = bass.AP(
                tensor=CONST[:].tensor,
                offset=CONST[:].offset,
                ap=[[0, P]] + list(CONST[:].ap[1:]),
            )
            nc.sync.dma_start(out=const_sb, in_=const_bcast)
            acc = singles.tile([P, 1], f32)
            nc.vector.memset(acc, 0.0)

            def row_tile(i):
                xt_t = xt_pool.tile([P, DC, P], f32r)
                for c in range(DC):
                    nc.sync.dma_start(
                        out=xt_t[:, c, :],
                        in_=XT[c * P:(c + 1) * P, bass.ds(i * P, P)],
                    )

                halfsq = small_pool.tile([P, K], f32, tag="halfsq")
                wlp = small_pool.tile([P, K], f32, tag="wlp")

                # x . c_k  (plus later the -halfsq): [128, 64]
                pt = pt_pool.tile([P, K], f32)
                for c in range(DC):
                    nc.tensor.matmul(
                        out=pt,
                        lhsT=xt_t[:, c, :],
                        rhs=c_sb[:, c, :],
                        start=(c == 0),
                        stop=(c == DC - 1),
                    )

                # comps [0, KS) -> ScalarE fused square+accum
                # comps [KS, K) -> VectorE bn_stats/bn_aggr (sum sq = D*(var+mean^2))
                KS = 40
                NDV = K - KS
                mv_t = small_pool.tile([P, NDV, 2], f32, tag="mv")

                for p in range(NPAIR):
                    ps = ppair_pool.tile([P, 2 * D], f32)
                    for c in range(DC):
                        nc.tensor.matmul(
                            out=ps,
                            lhsT=xt_t[:, c, :],
                            rhs=b_sb[:, c, 2 * D * p:2 * D * (p + 1)],
                            start=(c == 0),
                            stop=(c == DC - 1),
                        )
                    for s in range(2):
                        k = 2 * p + s
                        y_ap = ps[:, s * D:(s + 1) * D]
                        if k < KS:
                            s_scr = scr_pool.tile([P, D], f32, tag="s_scr")
                            nc.scalar.activation(
                                out=s_scr, in_=y_ap, func=AF.Square,
                                accum_out=halfsq[:, k:k + 1],
                            )
                        else:
                            stats = scr_pool.tile([P, 6], f32, tag="stats")
                            nc.vector.bn_stats(out=stats, in_=y_ap)
                            nc.vector.bn_aggr(out=mv_t[:, k - KS, :], in_=stats)

                # batched tail for the bn_stats block: sum_sq = D*(var + mean^2)
                msq = small_pool.tile([P, NDV], f32, tag="msq")
                nc.vector.tensor_tensor(msq, mv_t[:, :, 0], mv_t[:, :, 0], ALU.mult)
                nc.vector.tensor_tensor(msq, msq, mv_t[:, :, 1], ALU.add)
                nc.vector.tensor_scalar(
                    out=halfsq[:, KS:K], in0=msq, scalar1=float(D), scalar2=None,
                    op0=ALU.mult,
                )

                # wlp = pt - 0.5*sq + const
                nc.vector.tensor_scalar(
                    out=wlp, in0=halfsq, scalar1=-0.5, scalar2=None, op0=ALU.mult,
                )
                nc.vector.tensor_tensor(wlp, wlp, pt, ALU.add)
                nc.vector.tensor_tensor(wlp, wlp, const_sb, ALU.add)

                # logsumexp over the 64 components
                neg_rmax = small_pool.tile([P, 1], f32, tag="nrm")
                nc.vector.tensor_reduce(
                    out=neg_rmax, in_=wlp, axis=mybir.AxisListType.X,
                    op=ALU.max, negate=True,
                )
                e_scr = small_pool.tile([P, K], f32, tag="escr")
                sumexp = small_pool.tile([P, 1], f32, tag="sume")
                nc.scalar.activation(
                    out=e_scr, in_=wlp, func=AF.Exp,
                    bias=neg_rmax[:, 0:1], accum_out=sumexp,
                )
                # neg_lpn = neg_rmax - lse ;  acc += neg_lpn (host negates)
                lse_t = small_pool.tile([P, 1], f32, tag="lse")
                nc.scalar.activation(out=lse_t, in_=sumexp, func=AF.Ln)
                neg_lpn = small_pool.tile([P, 1], f32, tag="nlpn")
                nc.vector.tensor_tensor(neg_lpn, neg_rmax, lse_t, ALU.subtract)
                nc.vector.tensor_tensor(acc, acc, neg_lpn, ALU.add)

                resp_t = resp_pool.tile([P, K], f32)
                nc.vector.tensor_scalar(
                    out=resp_t, in0=wlp, scalar1=neg_lpn[:, 0:1], scalar2=None,
                    op0=ALU.add,
                )
                nc.sync.dma_start(out=RESP[bass.ds(i * P, P), :], in_=resp_t)

            with tc.For_i(0, NT, 1) as i:
                row_tile(i)

            nc.sync.dma_start(out=LPN[:], in_=acc)

    nc.compile()
    return nc


def _host_prep(X, weights, means, precision_cholesky):
    L = np.asarray(precision_cholesky, np.float64)
    mu = np.asarray(means, np.float64)
    w = np.asarray(weights, np.float64)

    t = np.einsum("kd,kdj->kj", mu, L)                       # mu_k @ L_k
    cvec = np.einsum("kdj,kj->kd", L, t)                     # L_k @ t_k
    logdet = np.sum(np.log(np.diagonal(L, axis1=1, axis2=2)), axis=1)
    const = (logdet + np.log(w)
             - 0.5 * (D * np.log(2.0 * np.pi) + np.sum(t * t, axis=1)))

    B = np.ascontiguousarray(
        np.transpose(L, (1, 0, 2)).reshape(D, K * D)).astype(np.float32)
    Bq = np.ascontiguousarray(B.reshape(DC, P, K * D))
    Cm = np.ascontiguousarray(cvec.T.astype(np.float32).reshape(DC, P, K))
    const32 = np.ascontiguousarray(const.astype(np.float32).reshape(1, K))

    X = np.asarray(X, np.float32)
    xts = [np.ascontiguousarray(X[i * NS:(i + 1) * NS].T) for i in range(NCORES)]
    return xts, Bq, Cm, const32


def kernel(X, weights, means, precision_cholesky):
    from concourse.bass_utils import run_bass_kernel_spmd

    if "nc" not in _CACHE:
        _CACHE["nc"] = _build_program()
    nc = _CACHE["nc"]

    xts, Bq, Cm, const32 = _host_prep(X, weights, means, precision_cholesky)
    in_maps = [
        {"XT": xts[i], "B": Bq, "C": Cm, "CONST": const32}
        for i in range(NCORES)
    ]
    out = run_bass_kernel_spmd(nc, in_maps, list(range(NCORES)))
    resp = np.concatenate([r["RESP"] for r in out.results], axis=0)
    neg_lpn_sum = float(sum(np.asarray(r["LPN"], np.float64).sum()
                            for r in out.results))
    mean = np.float32(-neg_lpn_sum / N)
    return mean, resp
